# revision 31
# baseline (speedup 1.0000x reference)
"""ColBERT pairwise scoring kernel for 8x TRN2 NeuronCores.

Computation (see problem reference):
    qn = l2norm(q, axis=-1); kn = l2norm(k, axis=-1)
    S[b,o,i,j] = qn[b,i,:]·kn[o,j,:], masked positions -> -inf
    s[b,o] = sum_i logsumexp_j(ALPHA*S)/ALPHA, nonfinite -> 0
    out = s / (sqrt(Lq*Lk)+1e-6) * min(exp(logit_scale), 100)

Sharding: candidate axis O is split across the 8 cores (16 o's per core);
q is replicated. Host pre-normalizes q and k, zeroes masked k rows (so
exp contributes exactly 1.0 there; the per-o masked count is subtracted
inside the final Ln bias), and sends both TRANSPOSED (d on partitions) so
the device does no transposes at all.

Per core, for each j-chunk (128 k rows) x bi-half (1024 query rows):
  - PE matmul (float32r, 1 cyc/row): T[j?, no: bi on free] = kt_chunk^T @ qt
    -> T [128 j, 1024 bi] in PSUM
  - ACT exp (scale=ALPHA): e = exp(ALPHA*T) -> SBUF bf16
  - PE reduce matmuls (bf16, 1 cyc/row): one-hot-column indicator weights
    accumulate sum_j e into plse[128, 256] where partition = o*8 + bihi,
    free = bi low 8 bits. All 256 reduce matmuls form one PSUM
    accumulation group in a single bank.
Tail: Ln(plse - nmasked + 1e-30) on ACT, sum over Lq (innermost 32) on
DVE, DMA out [128, 8].

Since |ALPHA*S| <= 12, no max-subtraction is needed for a stable logsumexp.
"""

import math
import sys
from contextlib import ExitStack

import numpy as np

for _p in ("/opt/trn_rl_repo",):
    if _p not in sys.path:
        sys.path.insert(0, _p)

import concourse.bass as bass
import concourse.bacc as bacc
import concourse.tile as tile
from concourse import mybir
from concourse.bass_utils import run_bass_kernel_spmd

ALPHA = 12.0
B, Lq, O, Lk, D = 64, 32, 128, 256, 128
NCORES = 8
BI = B * Lq  # 2048 query rows, replicated on every core

# DVE fast-exp (Schraudolph on bf16 bit patterns):
#   bf16_bits(e^y) ~= y * 128/ln(2) + (127*128 - C_CORR)
# The DVE computes bits = T*EXP_SLOPE + EXP_OFF as an int16 tensor_scalar
# (T = S, y = ALPHA*S), which is then bitcast to bf16 for the reduce
# matmul. C_CORR centers the piecewise-linear error (+-4.3%).
EXP_SLOPE = ALPHA * 184.66496234120901  # ALPHA * 2^7/ln2
C_CORR = 5.51
EXP_OFF = 16256.0 - C_CORR
# Which main-loop half-chunks the DVE handles (rest go to ACT exp):
# 28 of 64, Bresenham-spread
DVE_EXP = frozenset(
    it for it in range(64) if ((it + 1) * 28) // 64 > (it * 28) // 64)
# o's whose two j-chunks are pair-summed on DVE before a single (halved)
# PE reduce pass
PAIRED_O = frozenset({3, 7, 11, 15})

F32 = mybir.dt.float32
F16 = mybir.dt.float16
BF16 = mybir.dt.bfloat16
I16 = mybir.dt.int16
F8 = mybir.dt.float8e4
AF = mybir.ActivationFunctionType
OP = mybir.AluOpType
DR = mybir.MatmulPerfMode.DoubleRow


def emit_kernel(ctx, tc, qt_d, kt_d, out_d, OL):
    """Emit the per-core program. OL = number of o's on this core (16)."""
    nc = tc.nc
    KR = OL * Lk          # 4096 k rows on this core
    NCH = KR // 128       # 32 j-chunks
    NIT = NCH * 2         # 64 iterations: (chunk, bi-half)

    sing = ctx.enter_context(tc.tile_pool(name="sing", bufs=1))
    epool = ctx.enter_context(tc.tile_pool(name="epool", bufs=5))
    edpool = ctx.enter_context(tc.tile_pool(name="edpool", bufs=4))
    espool = ctx.enter_context(tc.tile_pool(name="espool", bufs=2))
    pm = ctx.enter_context(tc.tile_pool(name="pm", bufs=3, space="PSUM"))
    plp = ctx.enter_context(tc.tile_pool(name="plp", bufs=1, space="PSUM"))
    wp = ctx.enter_context(tc.tile_pool(name="wp", bufs=1, space="PSUM"))

    # fp8 DoubleRow layout: [Ki=64 partitions, Ko=2 k-tiles, cols];
    # element (p, t, col) holds dimension d = t*64 + p.
    qt = sing.tile([64, 2 * BI], F8)   # normalized q^T fp8 [p, (t bi)]
    kt = sing.tile([64, 2 * KR], F8)   # normalized masked k^T fp8 [p, (t j)]
    W = sing.tile([128, 256], BF16)    # indicator: col 128 = ones
    ssum = sing.tile([128, 256], F32)  # plse staging for DMA out
    qtr = qt.rearrange("p (t n) -> p t n", t=2)
    ktr = kt.rearrange("p (t n) -> p t n", t=2)

    # ---- inputs on two HWDGE queues (SP, ACT) with 3D APs that fetch
    # both k-tiles of a column range in one DMA ----
    qt3_d = qt_d.rearrange("p (t n) -> p t n", t=2)
    kt3_d = kt_d.rearrange("p (t n) -> p t n", t=2)
    nc.sync.dma_start(out=ktr[:, :, 0:256], in_=kt3_d[:, :, 0:256])
    nc.scalar.dma_start(out=qtr[:, :, 0:1024], in_=qt3_d[:, :, 0:1024])
    nc.sync.dma_start(out=qtr[:, :, 1024:2048], in_=qt3_d[:, :, 1024:2048])
    nc.scalar.dma_start(out=ktr[:, :, 256:2048], in_=kt3_d[:, :, 256:2048])
    nc.sync.dma_start(out=ktr[:, :, 2048:4096], in_=kt3_d[:, :, 2048:4096])

    nc.vector.memset(W, 0.0)
    nc.vector.memset(W[:, 128:129], 1.0)

    plse = plp.tile([128, 256], F32)

    # ---- PE p-state warmup: junk matmuls during the DMA fill ----
    junk = wp.tile([128, 128], F32)
    for _ in range(8):
        nc.tensor.matmul(out=junk, lhsT=W[:, 0:128], rhs=W[:, 0:128],
                         start=True, stop=True, skip_group_check=True)

    # ---- main loop, software-pipelined 2 deep:
    #      matmul(n) ... exp(n-1) ... reduce(n-2)
    Tt = {}
    et = {}
    for it in range(NIT + 2):
        if it < NIT:
            ch = it // 2          # j-chunk (o = ch // 2)
            h = it % 2            # bi half
            T = pm.tile([128, 1024], F32, tag="mm")
            for s in range(2):
                nc.tensor.matmul(
                    out=T[:, s * 512:(s + 1) * 512],
                    lhsT=ktr[:, :, ch * 128:(ch + 1) * 128],
                    rhs=qtr[:, :, h * 1024 + s * 512: h * 1024 + (s + 1) * 512],
                    start=True, stop=True, perf_mode=DR,
                )
            Tt[it] = T
        if 0 < it <= NIT:
            p = it - 1
            T = Tt.pop(p)
            if p in DVE_EXP:
                ed = edpool.tile([128, 1024], I16, tag="ed")
                nc.vector.tensor_scalar(
                    out=ed, in0=T, scalar1=float(EXP_SLOPE),
                    scalar2=float(EXP_OFF), op0=OP.mult, op1=OP.add)
                et[p] = ed.bitcast(BF16)
            else:
                e = epool.tile([128, 1024], BF16, tag="e")
                nc.scalar.activation(out=e, in_=T, func=AF.Exp,
                                     bias=0.0, scale=float(ALPHA))
                et[p] = e
        if it > 1:
            p = it - 2
            ch = p // 2
            h = p % 2
            o = ch // 2
            jc = ch % 2
            if o in PAIRED_O:
                if jc == 0:
                    continue  # wait for the o's second chunk
                eA = et.pop(4 * o + h)
                eB = et.pop(4 * o + 2 + h)
                es = espool.tile([128, 1024], BF16, tag="es")
                with nc.allow_low_precision(reason="bf16 pair sum"):
                    nc.vector.tensor_tensor(out=es, in0=eA, in1=eB, op=OP.add)
                e = es
            else:
                e = et.pop(p)
            for hb in range(4):
                pp = o * 8 + h * 4 + hb   # target partition in plse
                nc.tensor.matmul(
                    out=plse,
                    lhsT=W[:, 128 - pp:256 - pp],
                    rhs=e[:, hb * 256:(hb + 1) * 256],
                    start=(p == 0 and hb == 0),
                    stop=(p == NIT - 1 and hb == 3),
                )

    # ---- tail: ship the raw exp-sums; ln + Lq-sum happen on the host ----
    # (out-DMA issued from the ACT queue: its issue cost overlaps the
    # trailing reduce matmuls since ACT finishes first)
    nc.vector.tensor_copy(out=ssum, in_=plse)
    nc.scalar.dma_start(out=out_d, in_=ssum)


def build_program(OL):
    KR = OL * Lk
    nc = bacc.Bacc("TRN2", target_bir_lowering=False, debug=False,
                   enable_asserts=False, num_devices=NCORES)
    qt_d = nc.dram_tensor("qt_in", [64, 2 * BI], F8, kind="ExternalInput").ap()
    kt_d = nc.dram_tensor("kt_in", [64, 2 * KR], F8, kind="ExternalInput").ap()
    out_d = nc.dram_tensor("outp", [128, 256], F32, kind="ExternalOutput").ap()

    with tile.TileContext(nc) as tc, ExitStack() as ctx:
        emit_kernel(ctx, tc, qt_d, kt_d, out_d, OL)
    nc.compile()
    return nc


def make_in_maps(q, k, k_mask, OL, ncores):
    """Host-side shard prep. Returns per-core input dicts."""
    import ml_dtypes
    F8NP = ml_dtypes.float8_e4m3

    qf = np.asarray(q, dtype=np.float32).reshape(BI, D)
    qn = qf / np.maximum(np.sqrt((qf * qf).sum(-1, keepdims=True)), 1e-12)
    # DoubleRow pack: [p, t, bi] holds qn[bi, t*64+p]
    qt8 = np.ascontiguousarray(
        qn.T.reshape(2, 64, BI).transpose(1, 0, 2).reshape(64, 2 * BI)
    ).astype(F8NP)

    kf = np.asarray(k, dtype=np.float32).reshape(O * Lk, D)
    kn = kf / np.maximum(np.sqrt((kf * kf).sum(-1, keepdims=True)), 1e-12)
    km = np.asarray(k_mask).astype(bool).reshape(O * Lk)
    kn[km] = 0.0
    ktf = kn.T.reshape(2, 64, O * Lk).transpose(1, 0, 2)  # [p, t, OLk] f32

    in_maps = []
    for c in range(ncores):
        kt8 = np.ascontiguousarray(
            ktf[:, :, c * OL * Lk:(c + 1) * OL * Lk].reshape(64, 2 * OL * Lk)
        ).astype(F8NP)
        in_maps.append({
            "qt_in": qt8,
            "kt_in": kt8,
        })
    return in_maps


def postprocess(per_core_out, q_mask, k_mask, logit_scale, OL, ncores):
    """Gather per-core [128, 256] exp-sums into the final [B, O] output.

    Core c, partition p = o*8 + bihi, free f = bilo: value =
    sum_j exp(ALPHA*S) over this o's 256 j's for bi = bihi*256 + f.
    Host does: ln(sum - n_masked), sum over i (=f%32), reorder, scale.
    """
    # A masked k token contributes exactly 1.0 through the ACT exp path and
    # exactly V_DVE through the DVE bit-trick path; subtract per (o, h).
    V_DVE = 0.9765625  # bf16 bits int(EXP_OFF) = 16250
    kmc = np.asarray(k_mask).astype(bool).reshape(O, 2, 128).sum(-1)  # [O, jc]
    corr = np.zeros((O, 2), dtype=np.float64)  # [o, h]
    for ol in range(OL):
        for jc in range(2):
            for h in range(2):
                it = (ol * 2 + jc) * 2 + h
                v = V_DVE if it in DVE_EXP else 1.0
                for c in range(ncores):
                    corr[c * OL + ol, h] += kmc[c * OL + ol, jc] * v
    s = np.empty((B, ncores * OL), dtype=np.float32)
    with np.errstate(divide="ignore", invalid="ignore"):
        for c in range(ncores):
            r = np.asarray(per_core_out[c]).reshape(OL, 8, 8, Lq)  # [o,bihi,g,i]
            cc = corr[c * OL:(c + 1) * OL].reshape(OL, 2, 1, 1, 1)
            rr = r.reshape(OL, 2, 4, 8, Lq) - cc  # bihi = h*4 + hb
            lse = np.log(np.maximum(rr.reshape(OL, 8, 8, Lq), 1e-30))
            sd = lse.sum(axis=3).reshape(OL, B)  # b = bihi*8 + g
            s[:, c * OL:(c + 1) * OL] = sd.T
    coef = min(math.exp(float(logit_scale)), 100.0) / (
        ALPHA * (math.sqrt(Lq * Lk) + 1e-06))
    s = s * np.float32(coef)
    # rows with any masked query token are -inf in the reference -> zeroed
    s[np.asarray(q_mask).astype(bool).any(axis=1), :] = 0.0
    # fully-masked candidates are -inf in the reference -> zeroed
    s[:, np.asarray(k_mask).astype(bool).all(axis=1)] = 0.0
    s = np.where(np.isfinite(s), s, 0.0).astype(np.float32)
    return s


_CACHED_NC = None


def kernel(q, k, q_mask, k_mask, logit_scale):
    global _CACHED_NC
    OL = O // NCORES
    if _CACHED_NC is None:
        _CACHED_NC = build_program(OL)
    in_maps = make_in_maps(np.asarray(q), np.asarray(k), np.asarray(k_mask), OL, NCORES)
    res = run_bass_kernel_spmd(_CACHED_NC, in_maps, list(range(NCORES)))
    outs = [np.asarray(res.results[c]["outp"]) for c in range(NCORES)]
    return postprocess(outs, q_mask, k_mask, logit_scale, OL, NCORES)
